# revision 77
# baseline (speedup 1.0000x reference)
"""Trainium2 Bass kernel for nn_AttentionBlock (GroupNorm -> QKV 1x1 -> spatial
self-attention -> out-proj + residual), sharded over 8 NeuronCores.

Sharding: data-parallel over batch (2) x query-block (4). Each core gets its
batch image with pixel columns rolled so its 1024 queries are columns 0:1024
(attention + GroupNorm are permutation-invariant over key pixels), computes
K/V over all 4096 keys, and emits its (512, 1024) output slice.

Numerics: all matmuls in bf16/fp8 with f32 PSUM accumulation; GroupNorm stats,
softmax normalization and residual in f32.  bk is dropped (additive per-query
score constant, softmax-invariant); bv is folded into the output-proj bias
(softmax rows sum to 1), so bo_eff = bo + wo @ bv.  The attention scale is
applied inside the exp activation (fp8 wq cannot absorb it without underflow).

Schedule notes vs the first version:
 - x DMA split across the Sync and GpSimd hardware queues; GN stats are
   aggregated per channel-tile while later tiles stream in, so the
   scale/shift constants are ready ~1us after the last x byte lands.
 - PE warmup (keeps the HAM clock at full rate) is split into blocks sized
   to end exactly when the GN constants are ready.
 - K PSUM->SBUF fp8 casts run on Vector, V casts on GpSimd, Q bias-adds on
   GpSimd: no single engine's drain rate throttles the projection matmuls.
 - Q projection uses fp8 DoubleRow like K/V (reusing the same fp8 h tiles).
 - scores/proj share a 3-buffer PSUM pool so the exp ping-pong and the
   qc0->qc1 projection handoff do not stall the PE.
 - softmax denominator reciprocal uses the fast approx (~18 bits), the r
   accumulation chain retires before the U chains, and the output
   bias+residual is one fused affine_then_add per tile.
"""

import numpy as np
import ml_dtypes

import concourse.bass as bass
import concourse.bacc as bacc
import concourse.mybir as mybir
import concourse.tile as tile

F32 = mybir.dt.float32
BF16 = mybir.dt.bfloat16
FP8 = mybir.dt.float8e4
DR = mybir.MatmulPerfMode.DoubleRow
AF = mybir.ActivationFunctionType
ALU = mybir.AluOpType

P = 128
C = 512          # channels
CT = C // P      # 4 channel tiles
NK = 4096        # key pixels per batch image
KT = NK // P     # 32 key tiles
NQ = 1024        # queries per core
FD = 512         # matmul free-dim chunk
NCH = NK // FD   # 8 column chunks
G = 32           # groups
GS = C // G      # 16 channels per group
EPS = 1e-5
SCALE = float(C) ** -0.5
N_CORES = 8

# PE warmup blocks: 1 spans the x DMA, 2 bridges the group-stat chain,
# 3 bridges the A/B broadcast chain.  Calibrated from the trace.
NWARM1 = 134
NWARM2 = 5
NWARM3 = 12


def build_bass():
    nc = bacc.Bacc("TRN2", target_bir_lowering=False, debug=False,
                   num_devices=N_CORES)

    x_d = nc.dram_tensor("x", (C, NK), F32, kind="ExternalInput").ap()
    wq_d = nc.dram_tensor("wqT", (P, CT // 2, 2, C), FP8, kind="ExternalInput").ap()
    wk_d = nc.dram_tensor("wkT", (P, CT // 2, 2, C), FP8, kind="ExternalInput").ap()
    wv_d = nc.dram_tensor("wvT", (P, CT // 2, 2, C), FP8, kind="ExternalInput").ap()
    wo_d = nc.dram_tensor("woT", (CT, P, C), FP8, kind="ExternalInput").ap()
    bqs_d = nc.dram_tensor("bqs", (P, CT), F32, kind="ExternalInput").ap()
    boe_d = nc.dram_tensor("boe", (P, CT), F32, kind="ExternalInput").ap()
    gam_d = nc.dram_tensor("gam", (P, CT), F32, kind="ExternalInput").ap()
    bet_d = nc.dram_tensor("bet", (P, CT), F32, kind="ExternalInput").ap()
    indf_d = nc.dram_tensor("indf", (P, CT, G), F32, kind="ExternalInput").ap()
    indb_d = nc.dram_tensor("indb", (P, CT, P), F32, kind="ExternalInput").ap()
    out_d = nc.dram_tensor("out", (C, NQ), F32, kind="ExternalOutput").ap()

    with tile.TileContext(nc) as tc:
        with (
            tc.tile_pool(name="px", bufs=1) as px,
            tc.tile_pool(name="pw", bufs=1) as pw,
            tc.tile_pool(name="pc", bufs=1) as pcst,
            tc.tile_pool(name="ph", bufs=3) as ph,
            tc.tile_pool(name="pkvq", bufs=1) as pkvq,
            tc.tile_pool(name="pe", bufs=6) as pe,
            tc.tile_pool(name="psm", bufs=2) as psm,
            tc.tile_pool(name="po", bufs=3) as po,
            tc.tile_pool(name="ps_u", bufs=4, space="PSUM") as ps_u,
            tc.tile_pool(name="ps_s", bufs=3, space="PSUM") as ps_s,
            tc.tile_pool(name="ps_r", bufs=1, space="PSUM") as ps_r,
        ):
            # ---- x gets the Sync queue to itself (8KB lines, full rate);
            # small-line constants go on the Scalar queue in parallel and
            # weights follow x on Sync.
            x_sb = px.tile([P, CT, NK], F32, tag="x")
            XDH = 2048
            for ct in range(CT):
                rows = slice(ct * P, (ct + 1) * P)
                for xc in range(NK // XDH):
                    xcols = slice(xc * XDH, (xc + 1) * XDH)
                    nc.sync.dma_start(out=x_sb[:, ct, xcols],
                                      in_=x_d[rows, xcols])

            bqs_sb = pcst.tile([P, CT], F32, tag="bqs")
            nc.scalar.dma_start(out=bqs_sb, in_=bqs_d)
            boe_sb = pcst.tile([P, CT], F32, tag="boe")
            nc.scalar.dma_start(out=boe_sb, in_=boe_d)
            gam_sb = pcst.tile([P, CT], F32, tag="gam")
            nc.scalar.dma_start(out=gam_sb, in_=gam_d)
            bet_sb = pcst.tile([P, CT], F32, tag="bet")
            nc.scalar.dma_start(out=bet_sb, in_=bet_d)
            indf_sb = pcst.tile([P, CT, G], F32, tag="indf")
            nc.scalar.dma_start(out=indf_sb, in_=indf_d)
            indb_sb = pcst.tile([P, CT, P], F32, tag="indb")
            nc.scalar.dma_start(out=indb_sb, in_=indb_d)
            eps_sb = pcst.tile([P, 1], F32, tag="eps")
            nc.vector.memset(eps_sb, EPS)
            bqss_sb = pcst.tile([P, CT], F32, tag="bqss")
            nc.scalar.activation(out=bqss_sb, in_=bqs_sb, func=AF.Copy,
                                 scale=SCALE)

            # Weight loads are staged behind x: DMA descriptors stripe across
            # the hardware engines in dispatch order, so weight traffic in
            # flight steals engine slots and pushes x's last descriptor to the
            # end of the whole stream.  A garbage copy from x's final chunk
            # into each weight tile makes the weight DMA an overwrite that
            # cannot start before all of x has landed.
            wk_sb = pw.tile([P, CT // 2, 2, C], FP8, tag="wk")
            wv_sb = pw.tile([P, CT // 2, 2, C], FP8, tag="wv")
            wq_sb = pw.tile([P, CT // 2, 2, C], FP8, tag="wq")
            wo_sb = pw.tile([P, CT, C], FP8, tag="wo")
            stage_srcs = [x_sb[:, CT - 1, XDH + i * 4:XDH + (i + 1) * 4]
                          for i in range(4)]
            for i, (wt, part) in enumerate((
                    (wk_sb, wk_sb[:, 0, 0, 0:4]), (wv_sb, wv_sb[:, 0, 0, 0:4]),
                    (wq_sb, wq_sb[:, 0, 0, 0:4]), (wo_sb, wo_sb[:, 0, 0:4]))):
                nc.gpsimd.tensor_copy(out=part, in_=stage_srcs[i])
                if wt is wo_sb:
                    for ct in range(CT):
                        nc.sync.dma_start(out=wo_sb[:, ct, :], in_=wo_d[ct])
                else:
                    nc.sync.dma_start(out=wt, in_=(wk_d, wv_d, wq_d)[i])

            # ---- PE warmup block 1: spans the x DMA so the HAM clock-gate
            # stays at K=8/8 (2.4 GHz) when the real matmuls begin.
            ones_sb = pcst.tile([P, P], BF16, tag="ones")
            nc.vector.memset(ones_sb, 1.0)
            warm_rhs = pcst.tile([P, FD], BF16, tag="wrm")
            nc.vector.memset(warm_rhs, 0.0)
            wsink = pcst.tile([P, 1], F32, tag="wsink")

            def warm_block(n, name, drain=True):
                # drains on Scalar: a drain on Vector lands mid-bn_stats in
                # its in-order stream and stalls the GN statistics until the
                # PE warmup completes
                wps = ps_s.tile([P, FD], F32, tag="s", name=name)
                for i in range(n):
                    nc.tensor.matmul(wps, ones_sb, warm_rhs,
                                     start=(i == 0), stop=(i == n - 1))
                if drain:
                    nc.scalar.copy(out=wsink, in_=wps[:, 0:1])
                return wps

            warm_block(NWARM1, "warm1")

            # preload the Sqrt activation table while the DMA streams (the
            # load is ~1.3us and otherwise lands on the GN critical chain)
            tbl_sink = pcst.tile([P, 1], F32, tag="tbl")
            nc.scalar.activation(out=tbl_sink, in_=eps_sb, func=AF.Sqrt)

            # ---- GroupNorm statistics: bn_stats per 512-col chunk trailing
            # the DMA on Vector, aggregated per channel-tile as it completes.
            stats = pcst.tile([P, CT, NCH, 6], F32, tag="stats")
            mv = pcst.tile([P, CT, 2], F32, tag="mv")
            cstats = pcst.tile([P, CT, 2], F32, tag="cstats")
            for ct in range(CT):
                for s in range(NCH):
                    nc.vector.bn_stats(out=stats[:, ct, s, :],
                                       in_=x_sb[:, ct, s * FD:(s + 1) * FD])
                nc.vector.bn_aggr(out=mv[:, ct, :], in_=stats[:, ct])
                # cstats = [mean_c, var_c + mean_c^2] on GpSimd (keeps the
                # Vector queue free for bn_stats and the Scalar activation
                # table holding Sqrt)
                nc.gpsimd.tensor_tensor(cstats[:, ct, 1:2], mv[:, ct, 0:1],
                                        mv[:, ct, 0:1], ALU.mult)
                nc.gpsimd.tensor_tensor(cstats[:, ct, 1:2], cstats[:, ct, 1:2],
                                        mv[:, ct, 1:2], ALU.add)
                nc.gpsimd.tensor_copy(out=cstats[:, ct, 0:1], in_=mv[:, ct, 0:1])

            # group combine: [32, 2] = sum_ct indf^T @ cstats  (weights 1/16)
            gps = ps_s.tile([G, 2], F32, tag="s", name="gps")
            for ct in range(CT):
                nc.tensor.matmul(gps, indf_sb[:, ct, :], cstats[:, ct, :],
                                 start=(ct == 0), stop=(ct == CT - 1))
            w2ps = warm_block(NWARM2, "warm2", drain=False)

            gsb = pcst.tile([P, 2], F32, tag="gsb")
            nc.vector.tensor_copy(out=gsb[0:G, :], in_=gps)
            # grhs = [mu_g, rstd_g], zero-padded to 128 partitions
            grhs = pcst.tile([P, 2], F32, tag="grhs")
            nc.vector.memset(grhs, 0.0)
            sq = pcst.tile([P, 1], F32, tag="sq")
            nc.gpsimd.tensor_tensor(sq[0:G], gsb[0:G, 0:1], gsb[0:G, 0:1],
                                    ALU.mult)
            nc.vector.tensor_tensor(sq[0:G], gsb[0:G, 1:2], sq[0:G], ALU.subtract)
            nc.scalar.activation(out=sq[0:G], in_=sq[0:G], func=AF.Sqrt,
                                 bias=eps_sb[0:G])
            nc.vector.tensor_copy(out=grhs[0:G, 0:1], in_=gsb[0:G, 0:1])
            nc.vector.reciprocal(out=grhs[0:G, 1:2], in_=sq[0:G])

            # broadcast to per-channel scale/shift: h = x*A + B
            A_sb = pcst.tile([P, CT], F32, tag="A")
            B_sb = pcst.tile([P, CT], F32, tag="B")
            abps = ps_s.tile([P, CT, 2], F32, tag="s", name="ab")
            for ct in range(CT):
                nc.tensor.matmul(abps[:, ct, :], indb_sb[:, ct, :], grhs,
                                 start=True, stop=True)
            w3ps = warm_block(NWARM3, "warm3", drain=False)
            # drain warm2/warm3 only now, after the Sqrt chain, so the scalar
            # queue does not gate the real Sqrt on the warmup blocks
            nc.scalar.copy(out=wsink, in_=w2ps[:, 0:1])
            nc.scalar.copy(out=wsink, in_=w3ps[:, 0:1])
            # preload the Exp table now that Sqrt is done (input from sq so
            # the scheduler cannot hoist it before the real Sqrt and evict
            # the Sqrt table; scalar is idle here)
            nc.scalar.activation(out=tbl_sink, in_=eps_sb, func=AF.Exp,
                                 bias=grhs[:, 1:2])
            for ct in range(CT):
                nc.vector.tensor_tensor(A_sb[:, ct:ct + 1], abps[:, ct, 1:2],
                                        gam_sb[:, ct:ct + 1], ALU.mult)
                nc.vector.tensor_tensor(B_sb[:, ct:ct + 1], abps[:, ct, 0:1],
                                        A_sb[:, ct:ct + 1], ALU.mult)
                nc.vector.tensor_tensor(B_sb[:, ct:ct + 1], bet_sb[:, ct:ct + 1],
                                        B_sb[:, ct:ct + 1], ALU.subtract)

            # ---- GN apply + Q/K/Vt projections, per 512-column chunk ----
            # K/Q/Vt stored as fp8 e4m3 in DoubleRow pair layout.
            # K and V chains are interleaved so their PSUM drains alternate
            # between Vector (K casts) and GpSimd (V casts, Q bias-adds).
            k_sb = pkvq.tile([P, CT // 2, 2, NK], FP8, tag="K")
            vt_sb = pkvq.tile([P, KT // 2, 2, FD], FP8, tag="Vt")
            q_sb = pkvq.tile([P, CT // 2, 2, NQ], FP8, tag="Q")
            onesp_sb = pcst.tile([P, 2, P], FP8, tag="onesp")
            nc.vector.memset(onesp_sb, 1.0)
            for ch in range(NCH):
                cols = slice(ch * FD, (ch + 1) * FD)
                h_ch = ph.tile([P, CT // 2, 2, FD], FP8, tag="h")
                for ct in range(CT):
                    # GN apply (SBUF->SBUF): h = x*A + B; GpSimd takes 3 of 4
                    eng = nc.vector if ct == 0 else nc.gpsimd
                    eng.tensor_scalar(
                        out=h_ch[:, ct // 2, ct % 2, :], in0=x_sb[:, ct, cols],
                        scalar1=A_sb[:, ct:ct + 1], scalar2=B_sb[:, ct:ct + 1],
                        op0=ALU.mult, op1=ALU.add)
                for j in range(CT):
                    # K chain j: K[ot=j, cols]
                    kps = ps_u.tile([P, FD], F32, tag="u", name=f"k{ch}_{j}")
                    for ctp in range(CT // 2):
                        nc.tensor.matmul(kps,
                                         wk_sb[:, ctp, :, j * P:(j + 1) * P],
                                         h_ch[:, ctp, :, :], perf_mode=DR,
                                         start=(ctp == 0),
                                         stop=(ctp == CT // 2 - 1))
                    nc.vector.tensor_copy(out=k_sb[:, j // 2, j % 2, cols],
                                          in_=kps)
                    # Vt chain j: Vt[kt = ch*4+j]
                    kt = ch * CT + j
                    vps = ps_u.tile([P, FD], F32, tag="u", name=f"v{ch}_{j}")
                    for ctp in range(CT // 2):
                        nc.tensor.matmul(vps,
                                         h_ch[:, ctp, :, j * P:(j + 1) * P],
                                         wv_sb[:, ctp, :, :], perf_mode=DR,
                                         start=(ctp == 0),
                                         stop=(ctp == CT // 2 - 1))
                    nc.scalar.copy(out=vt_sb[:, kt // 2, kt % 2, :], in_=vps)
                    # Q chain j (first 1024 columns only), bias split V/S
                    if ch < NQ // FD:
                        qps = ps_s.tile([P, FD], F32, tag="s",
                                        name=f"q{ch}_{j}")
                        for ctp in range(CT // 2):
                            nc.tensor.matmul(qps,
                                             wq_sb[:, ctp, :, j * P:(j + 1) * P],
                                             h_ch[:, ctp, :, :], perf_mode=DR,
                                             start=(ctp == 0),
                                             stop=(ctp == CT // 2 - 1))
                        # q = (qps + bq) * SCALE so exp needs no scale pass
                        if j % 2 == 0:
                            nc.vector.tensor_scalar(
                                out=q_sb[:, j // 2, j % 2, cols], in0=qps,
                                scalar1=bqs_sb[:, j:j + 1], scalar2=SCALE,
                                op0=ALU.add, op1=ALU.mult)
                        else:
                            nc.scalar.activation(
                                out=q_sb[:, j // 2, j % 2, cols], in_=qps,
                                func=AF.Identity, scale=SCALE,
                                bias=bqss_sb[:, j:j + 1])

            # ---- attention: St = K^T Q per k-tile, exp, U += Vt^T E, r += 1^T E
            attn_sb = pkvq.tile([P, CT, NQ], BF16, tag="attn")
            for qc in range(NQ // FD):
                qcols = slice(qc * FD, (qc + 1) * FD)
                u_ps = [ps_u.tile([P, FD], F32, tag="u", name=f"u{qc}_{cv}")
                        for cv in range(CT)]
                r_ps = ps_r.tile([P, FD], F32, tag="r")
                KTP = KT // 2
                pend = []

                def emit_u(ep, ktp, qc=qc, u_ps=u_ps, r_ps=r_ps):
                    last = (ktp == KTP - 1)
                    if last:
                        # retire r first so the reciprocal starts early
                        nc.tensor.matmul(r_ps, onesp_sb, ep, perf_mode=DR,
                                         start=(ktp == 0), stop=True)
                    for cv in range(CT):
                        nc.tensor.matmul(u_ps[cv],
                                         vt_sb[:, ktp, :, cv * P:(cv + 1) * P],
                                         ep, perf_mode=DR,
                                         start=(ktp == 0), stop=last)
                    if not last:
                        nc.tensor.matmul(r_ps, onesp_sb, ep, perf_mode=DR,
                                         start=(ktp == 0), stop=False)

                for ktp in range(KTP):
                    ep = pe.tile([P, 2, FD], FP8, tag="e", name=f"e{qc}_{ktp}")
                    for i in range(2):
                        kt = 2 * ktp + i
                        sps = ps_s.tile([P, FD], F32, tag="s", name=f"s{qc}_{kt}")
                        for ctp in range(CT // 2):
                            nc.tensor.matmul(sps,
                                             k_sb[:, ctp, :, kt * P:(kt + 1) * P],
                                             q_sb[:, ctp, :, qcols],
                                             perf_mode=DR,
                                             start=(ctp == 0),
                                             stop=(ctp == CT // 2 - 1))
                        nc.scalar.activation(out=ep[:, i, :], in_=sps,
                                             func=AF.Exp)
                    pend.append((ep, ktp))
                    if len(pend) > 2:
                        emit_u(*pend.pop(0))
                for item in pend:
                    emit_u(*item)

                # normalize U by the softmax denominator (fast approx recip),
                # casts split across Vector and GpSimd
                rr = psm.tile([P, FD], F32, tag="rr")
                nc.vector.reciprocal_approx_fast(out=rr, in_=r_ps)
                for cv in range(CT):
                    nc.vector.tensor_tensor(attn_sb[:, cv, qcols], u_ps[cv],
                                            rr, ALU.mult)

                # output projection + fused bias+residual for this q-chunk;
                # qc=0's projection overlaps qc=1's attention on the PE.
                for ot in range(CT):
                    ops = ps_s.tile([P, FD], F32, tag="s", name=f"proj{qc}_{ot}")
                    for cv in range(CT):
                        nc.tensor.matmul(ops,
                                         wo_sb[:, cv, ot * P:(ot + 1) * P],
                                         attn_sb[:, cv, qcols],
                                         start=(cv == 0), stop=(cv == CT - 1))
                    o_sb = po.tile([P, FD], F32, tag="o", name=f"o{qc}_{ot}")
                    rows = slice(ot * P, (ot + 1) * P)
                    if qc == 1 and ot >= CT - 2:
                        # last two tiles: halved affine + DMA for a shorter
                        # serial tail, split across two queues
                        half = FD // 2
                        for hf, eng in ((0, nc.sync), (1, nc.scalar)):
                            cl = slice(hf * half, (hf + 1) * half)
                            nc.vector.affine_then_add(
                                out=o_sb[:, cl], in0=ops[:, cl],
                                in1=x_sb[:, ot, qc * FD + hf * half:
                                         qc * FD + (hf + 1) * half],
                                scale=1.0, bias=boe_sb[:, ot:ot + 1])
                            eng.dma_start(
                                out=out_d[rows, qc * FD + hf * half:
                                          qc * FD + (hf + 1) * half],
                                in_=o_sb[:, cl])
                    else:
                        nc.vector.affine_then_add(out=o_sb, in0=ops,
                                                  in1=x_sb[:, ot, qcols],
                                                  scale=1.0,
                                                  bias=boe_sb[:, ot:ot + 1])
                        eng = nc.sync if ot % 2 == 0 else nc.scalar
                        eng.dma_start(out=out_d[rows, qcols], in_=o_sb)
    nc.compile()
    return nc


def make_core_inputs(x, gn_w, gn_b, wq, bq, wk, bk, wv, bv, wo, bo):
    """Build the 8 per-core input maps from full inputs."""
    bf16 = ml_dtypes.bfloat16
    f32 = np.float32
    b = x.shape[0]
    xf = np.ascontiguousarray(np.asarray(x, f32).reshape(b, C, NK))

    def wslice(w):
        wT = np.ascontiguousarray(
            np.asarray(w, f32).T.astype(ml_dtypes.float8_e4m3))
        return np.ascontiguousarray(wT.reshape(CT, P, C))

    woT = wslice(wo)

    def wpair(w):  # (512,512) w[o,c] -> fp8 pair layout [p, ctp, i, o]
        wT = np.asarray(w, f32).T.astype(ml_dtypes.float8_e4m3)
        return np.ascontiguousarray(
            wT.reshape(CT // 2, 2, P, C).transpose(2, 0, 1, 3))

    wqT, wkT, wvT = wpair(wq), wpair(wk), wpair(wv)

    def percol(v):  # (512,) -> (128, 4): [p, ct]
        return np.ascontiguousarray(np.asarray(v, f32).reshape(CT, P).T)

    # attention scale is applied inside the exp activation, not here
    bqs = percol(np.asarray(bq, f32))
    bo_eff = percol(np.asarray(bo, np.float64)
                    + np.asarray(wo, np.float64) @ np.asarray(bv, np.float64))
    gam = percol(gn_w)
    bet = percol(gn_b)

    indf = np.zeros((P, CT, G), f32)
    indb = np.zeros((P, CT, P), f32)
    for ct in range(CT):
        for p in range(P):
            g = (ct * P + p) // GS
            indf[p, ct, g] = 1.0 / GS
            indb[g, ct, p] = 1.0
    shared = dict(wqT=wqT, wkT=wkT, wvT=wvT, woT=woT, bqs=bqs, boe=bo_eff,
                  gam=gam, bet=bet, indf=indf, indb=indb)

    in_maps = []
    for core in range(N_CORES):
        bb, qb = core // 4, core % 4
        qs = qb * NQ
        xr = np.ascontiguousarray(
            np.concatenate([xf[bb][:, qs:], xf[bb][:, :qs]], axis=1))
        in_maps.append(dict(x=xr, **shared))
    return in_maps


_NC_CACHE = None


def _get_nc():
    global _NC_CACHE
    if _NC_CACHE is None:
        _NC_CACHE = build_bass()
    return _NC_CACHE


def run_on_cores(in_maps, **kw):
    from concourse.bass_utils import run_bass_kernel_spmd
    nc = _get_nc()
    return run_bass_kernel_spmd(nc, in_maps, core_ids=list(range(N_CORES)), **kw)


def kernel(**inputs):
    x = np.asarray(inputs["x"])
    b, c, H, W = x.shape
    in_maps = make_core_inputs(**inputs)
    res = run_on_cores(in_maps)
    out = np.zeros((b, C, NK), np.float32)
    for core in range(N_CORES):
        bb, qb = core // 4, core % 4
        out[bb][:, qb * NQ:(qb + 1) * NQ] = res.results[core]["out"]
    return out.reshape(b, c, H, W)
